# revision 31
# baseline (speedup 1.0000x reference)
"""2-layer GAT (DGL GATConv-style) on 8 TRN2 NeuronCores.

Strategy (host preprocessing is index/structure only; every FLOP that depends
on float inputs runs on device):
 - Nodes are dealt to 8 cores snake-wise by in-degree.  Self-loop edges are
   removed from the gather grid entirely (served by a per-window matmul from
   a per-core x_selfT input for layer 1, and local staged rows for layer 2).
 - Each node gets a parity (even/odd table position), chosen by local search
   so that every dst's in-edges split evenly between parities.  Gathers index
   PAIR-rows (stride 1536B, even/odd base offset), so the full 50k-node table
   fits the int16 index range without an A/B owner split.  Per-core windows
   are packed by max(e,o) so per-window slice counts are tight (pad ~1.14 vs
   1.41 for the old A/B scheme).
 - Layer-1 feature table is built locally on every core from a replicated
   xT input (no layer-1 AllGather).  The layer-2 table AllGather is split
   into 7 chunks overlapped with the layer-1 edge phase.
 - Table row: [256 feat dmaj | 4 ones | 4 el(bf16)] padded to 384 bf16 slots
   (768B, the 256B-granule ceiling).  Denominators ride along as the ones
   columns; el is consumed straight from the gathered rows.
 - Gather calls cover up to 12 slices (1536 rows) on 4 SWDGE queues
   round-robin with 4-deep G buffering to keep the dispatch pipes full.
"""
import sys
import types

import numpy as np
import ml_dtypes

import concourse.bass as bass
import concourse.bacc as bacc
import concourse.tile as tile
from concourse import mybir
from concourse.bass_utils import run_bass_kernel_spmd
from concourse.masks import make_identity

AF = mybir.ActivationFunctionType
ALU = mybir.AluOpType
BF16 = mybir.dt.bfloat16
F32 = mybir.dt.float32
I16 = mybir.dt.int16

P = 128
HEADS = 4
D = 64
FD = HEADS * D          # 256
ROW = 384               # bf16 slots per table row: 256 feat | 4 ones | 4 el | 120 pad
CORES = 8
NEG_SLOPE = 0.2
K_CAP = 24              # max slices per pass (SBUF bound)
CHUNK = 12              # slices per dma_gather call
NCHUNK = 7              # layer-2 AllGather chunks (must divide wpc)

LAST_EXEC_NS = None


def _install_profile_hook():
    """Best-effort NTFF profiling hook (axon images lack antenv.axon_hooks)."""
    try:
        import antenv
        try:
            import antenv.axon_hooks  # noqa: F401
            return
        except ImportError:
            pass
        mod = types.ModuleType("antenv.axon_hooks")
        mod._HOOK = None

        def set_hook(h):
            mod._HOOK = h

        def get_hook():
            return mod._HOOK

        mod.set_axon_ntff_profile_hook = set_hook
        mod.get_axon_ntff_profile_hook = get_hook
        sys.modules["antenv.axon_hooks"] = mod
        antenv.axon_hooks = mod
        from trn_agent_boot.trn_boot import _ntff_profile_via_ctypes
        set_hook(_ntff_profile_via_ctypes("/opt/axon/libaxon_pjrt.so"))
    except Exception:
        pass


def _dmaj(n):
    """column permutation h*64+d -> d*4+h (applied to axis of size 256)."""
    j = np.arange(n)
    d, h = j // HEADS, j % HEADS
    return h * D + d


def _wrap_idx(flat):
    """[128] int16 -> [128, 8] wrapped+replicated for dma_gather."""
    w = flat.reshape(8, 16).T  # [16, 8]
    return np.tile(w, (8, 1)).astype(np.int16)


def _prep(src, dst, n_nodes):
    """Host-side graph preprocessing: parity assignment, window packing,
    per-core index/mask buffers, per-layer row maps."""
    loop = src == dst
    nself = np.bincount(dst[loop], minlength=n_nodes)
    src_e, dst_e = src[~loop], dst[~loop]
    deg = np.bincount(dst_e, minlength=n_nodes)

    order = np.argsort(-deg, kind="stable")
    owner = np.empty(n_nodes, dtype=np.int32)
    pat = np.concatenate([np.arange(CORES), np.arange(CORES)[::-1]])
    owner[order] = pat[np.arange(n_nodes) % (2 * CORES)]

    npc = (n_nodes + CORES - 1) // CORES
    wpc = (npc + P - 1) // P
    assert wpc % NCHUNK == 0, wpc
    shard = wpc * P

    # ---- parity local search: balance each dst's in-edges over parities ----
    so = np.argsort(src_e, kind="stable")
    sstarts = np.zeros(n_nodes + 1, np.int64)
    np.cumsum(np.bincount(src_e, minlength=n_nodes), out=sstarts[1:])
    odf = dst_e[so]

    par = np.zeros(n_nodes, np.int8)
    for c in range(CORES):
        nodes = np.where(owner == c)[0]
        par[nodes] = np.arange(len(nodes)) % 2
    ecnt = np.bincount(dst_e, weights=par[src_e], minlength=n_nodes).astype(np.int64)

    def gain_of(n):
        ds = odf[sstarts[n]:sstarts[n + 1]]
        e, d = ecnt[ds], deg[ds]
        pm = 1 - 2 * int(par[n])
        return (np.maximum(e, d - e) - np.maximum(e + pm, d - e - pm)).sum()

    for sweep in range(10):
        e_of = ecnt[odf]
        d_of = deg[odf]
        pm = (1 - 2 * par[src_e[so]]).astype(np.int64)
        gsnap = np.zeros(n_nodes)
        np.add.at(gsnap, src_e[so], (np.maximum(e_of, d_of - e_of) -
                                     np.maximum(e_of + pm, d_of - e_of - pm)).astype(np.float64))
        swaps = 0
        for c in range(CORES):
            nodes = np.where(owner == c)[0]
            ones = nodes[par[nodes] == 1]
            zeros = nodes[par[nodes] == 0]
            o1 = ones[np.argsort(-gsnap[ones])]
            o0 = zeros[np.argsort(-gsnap[zeros])]
            i = j = 0
            while i < len(o1) and j < len(o0):
                u, v = o1[i], o0[j]
                gu, gv = gain_of(u), gain_of(v)
                if gu + gv <= 0:
                    if gsnap[u] <= gsnap[v]:
                        i += 1
                    else:
                        j += 1
                    if gsnap[u] <= 0 and gsnap[v] <= 0:
                        break
                    continue
                np.add.at(ecnt, odf[sstarts[u]:sstarts[u + 1]], -1)
                par[u] = 0
                np.add.at(ecnt, odf[sstarts[v]:sstarts[v + 1]], 1)
                par[v] = 1
                swaps += 1
                i += 1
                j += 1
        if swaps < 50:
            break

    # ---- window packing: per core sorted by max(e,o), 64/64 parity slots ----
    # partition v within a window is parity-interleaved (par-0 at even v), so
    # table position == w*128+v everywhere (device stages rows by partition).
    core_nodes = []           # per core: window-major list, padded with -1
    pos = np.full(n_nodes, -1, np.int64)
    KEs = np.zeros(wpc, np.int64)
    KOs = np.zeros(wpc, np.int64)
    for c in range(CORES):
        nodes = np.where(owner == c)[0]
        key = np.maximum(ecnt[nodes], deg[nodes] - ecnt[nodes])
        nodes = nodes[np.argsort(key, kind="stable")]
        win = np.full((wpc, P), -1, np.int64)
        wcap = np.full((wpc, 2), 64, np.int64)
        ptr = [0, 0]
        for n in nodes:
            p = int(par[n])
            while wcap[ptr[p], p] == 0:
                ptr[p] += 1
            w = ptr[p]
            v = 2 * (64 - wcap[w, p]) + p
            wcap[w, p] -= 1
            win[w, v] = n
            pos[n] = w * P + v
            KEs[w] = max(KEs[w], ecnt[n])
            KOs[w] = max(KOs[w], deg[n] - ecnt[n])
        core_nodes.append(win)

    rho = pos + owner.astype(np.int64) * shard        # layer-1 table row
    # layer-2 table row (chunk-major for chunked AllGather)
    csz = (wpc // NCHUNK) * P                         # rows per core per chunk
    chn = pos // csz
    rho2 = chn * (CORES * csz) + owner.astype(np.int64) * csz + pos % csz

    ks = np.stack([KEs, KOs], axis=1)
    passes = []
    for w in range(wpc):
        ke, ko = int(KEs[w]), int(KOs[w])
        cuts = []
        a0 = b0 = 0
        while a0 < ke or b0 < ko or not cuts:
            ta = min(ke - a0, K_CAP)
            tb = min(ko - b0, K_CAP - ta)
            cuts.append((a0, a0 + ta, b0, b0 + tb))
            a0 += ta
            b0 += tb
        passes.append(cuts)

    # ---- per-core slot buffers ----
    eo = np.argsort(dst_e, kind="stable")
    starts = np.zeros(n_nodes + 1, np.int64)
    np.cumsum(np.bincount(dst_e, minlength=n_nodes), out=starts[1:])

    sum_k = int((KEs + KOs).sum())
    idx1 = np.zeros((CORES, P, sum_k * 8), np.int16)
    idx2 = np.zeros((CORES, P, sum_k * 8), np.int16)
    masks = np.full((CORES, P, sum_k), -1e30, np.float32)
    # padded partitions get a phantom self-loop so their denominator is 1:
    # keeps h finite there (the cross-partition PE tree sum would otherwise
    # smear their NaNs over the whole window); host discards those rows.
    nselfw = np.ones((CORES, shard, HEADS), np.float32)
    for c in range(CORES):
        win = core_nodes[c]
        buf1 = np.zeros((sum_k, P), np.int16)
        buf2 = np.zeros((sum_k, P), np.int16)
        ck = 0
        for w in range(wpc):
            ke, ko = int(KEs[w]), int(KOs[w])
            for v in range(P):
                n = win[w, v]
                if n < 0:
                    continue
                nselfw[c, w * P + v, :] = nself[n]
                es = eo[starts[n]:starts[n + 1]]
                s = src_e[es]
                se = s[par[s] == 0]
                so_ = s[par[s] == 1]
                r1e, r1o = rho[se] // 2, rho[so_] // 2
                r2e, r2o = rho2[se] // 2, rho2[so_] // 2
                buf1[ck:ck + len(r1e), v] = r1e
                buf1[ck + ke:ck + ke + len(r1o), v] = r1o
                buf2[ck:ck + len(r2e), v] = r2e
                buf2[ck + ke:ck + ke + len(r2o), v] = r2o
                masks[c, v, ck:ck + len(r1e)] = 0.0
                masks[c, v, ck + ke:ck + ke + len(r1o)] = 0.0
            ck += ke + ko
        idx1[c] = np.concatenate([_wrap_idx(buf1[i]) for i in range(sum_k)], axis=1)
        idx2[c] = np.concatenate([_wrap_idx(buf2[i]) for i in range(sum_k)], axis=1)

    return dict(ks=ks, wpc=wpc, shard=shard, passes=passes, csz=csz,
                core_nodes=core_nodes, rho=rho, idx1=idx1, idx2=idx2,
                masks=masks, nselfw=nselfw, sum_k=sum_k)


def _build(ks, passes, wpc, shard, csz):
    """Build the SPMD bass program (identical on all cores)."""
    sum_k = int(ks.sum())
    kmax = int(ks.sum(axis=1).max())
    kpmax = max(max(a1 - a0 + b1 - b0 for (a0, a1, b0, b1) in cuts) for cuts in passes)
    full_rows = CORES * shard
    gshard = CORES * csz  # rows per chunk in the layer-2 table

    nc = bacc.Bacc("TRN2", target_bir_lowering=False, num_swdge_queues=4,
                   num_devices=CORES, dynamic_dma_scratch_size=16384)
    xta = nc.dram_tensor("xta", [D, full_rows], BF16, kind="ExternalInput")
    xsf = nc.dram_tensor("xsf", [D, shard], BF16, kind="ExternalInput")
    w1c = nc.dram_tensor("w1c", [P, 268], BF16, kind="ExternalInput")
    w2c = nc.dram_tensor("w2c", [2, P, 268], BF16, kind="ExternalInput")
    b1b = nc.dram_tensor("b1b", [P, FD], BF16, kind="ExternalInput")
    b2b = nc.dram_tensor("b2b", [P, FD], F32, kind="ExternalInput")
    idx1 = nc.dram_tensor("idx1", [P, sum_k * 8], I16, kind="ExternalInput")
    idx2 = nc.dram_tensor("idx2", [P, sum_k * 8], I16, kind="ExternalInput")
    mk = nc.dram_tensor("mk", [P, sum_k], F32, kind="ExternalInput")
    nsf = nc.dram_tensor("nsf", [shard, HEADS], F32, kind="ExternalInput")
    out = nc.dram_tensor("out", [shard, FD], F32, kind="ExternalOutput")

    qctr = [0]

    with tile.TileContext(nc) as tc, nc.allow_low_precision(reason="bf16 message accumulation is within tolerance"):
        with (
            tc.tile_pool(name="const", bufs=1) as cpool,
            tc.tile_pool(name="xt", bufs=2) as xtp,
            tc.tile_pool(name="sgrp", bufs=2) as sgp,
            tc.tile_pool(name="fpsum", bufs=2, space="PSUM") as fpsum,
            tc.tile_pool(name="tpsum", bufs=2, space="PSUM") as tpsum,
            tc.tile_pool(name="stage", bufs=3) as stp,
            tc.tile_pool(name="gat", bufs=5) as gatp,
            tc.tile_pool(name="trp", bufs=2, space="PSUM") as trpsum,
            tc.tile_pool(name="accp", bufs=2) as accp,
            tc.tile_pool(name="small", bufs=8) as smp,
            tc.tile_pool(name="widx", bufs=3) as wip,
            tc.tile_pool(name="ht", bufs=2) as htp,
            tc.tile_pool(name="dram", bufs=1, space="DRAM") as dram,
        ):
            ident = cpool.tile([P, P], BF16)
            make_identity(nc, ident[:])
            w1t = cpool.tile([P, 268], BF16)
            nc.sync.dma_start(w1t[:], w1c[:])
            w2t = [cpool.tile([P, 268], BF16, tag=f"w2_{i}", name=f"w2t{i}") for i in range(2)]
            nc.sync.dma_start(w2t[0][:], w2c[0])
            nc.sync.dma_start(w2t[1][:], w2c[1])
            b1t = cpool.tile([P, FD], BF16)
            nc.sync.dma_start(b1t[:], b1b[:])
            b2t = cpool.tile([P, FD], F32)
            nc.sync.dma_start(b2t[:], b2b[:])
            xst = cpool.tile([D, shard], BF16)
            nc.sync.dma_start(xst[:], xsf[:])

            tabs = [dram.tile([full_rows, ROW], BF16, tag=f"tab{l}", name=f"tab{l}")
                    for l in range(2)]
            tab_locs1 = [dram.tile([csz, ROW], BF16, tag=f"tl{i}", name=f"tl{i}")
                         for i in range(NCHUNK)]
            er_tab2 = dram.tile([shard, HEADS], F32)
            h_tab = dram.tile([shard, FD], BF16)

            # pair-row views of the tables (stride 1536B, even/odd base)
            tviews = []
            for l in range(2):
                t2 = tabs[l][:].rearrange("(r two) c -> r (two c)", two=2)
                tviews.append((t2[:, 0:ROW], t2[:, ROW:2 * ROW]))

            # ---------------- layer-1 table build (local, full table) --------
            GRP = [25, 24]  # chunks per write group within each shard
            wq = [nc.sync, nc.scalar]
            wqi = [0]
            for g in range(CORES):
                xt = xtp.tile([D, shard], BF16, tag="xt")
                nc.sync.dma_start(xt[:], xta[:, g * shard:(g + 1) * shard])
                i0 = 0
                for gi, ng in enumerate(GRP):
                    sg = sgp.tile([P, 25 * 264], BF16, tag="sg")
                    for i in range(ng):
                        ci = i0 + i
                        pf = fpsum.tile([P, 268], F32, tag="fp")
                        nc.tensor.matmul(pf[:], lhsT=xt[0:D, ci * P:(ci + 1) * P],
                                         rhs=w1t[0:D, :], start=True, stop=True)
                        if ci % 2:
                            nc.scalar.activation(sg[:, i * 264:i * 264 + 264],
                                                 pf[:, 0:264], AF.Copy)
                        else:
                            nc.vector.tensor_copy(sg[:, i * 264:i * 264 + 264],
                                                  pf[:, 0:264])
                    ones3 = sg[:, 0:ng * 264].rearrange("p (k r) -> p k r", r=264)
                    nc.vector.memset(ones3[:, :, FD:FD + 4], 1.0)
                    # write group -> tab rows [g*shard + i0*P, +ng*P), cols 0:264
                    dst3 = (tabs[0][g * shard + i0 * P:g * shard + (i0 + ng) * P, 0:264]
                            .rearrange("(k p) c -> p k c", p=P))
                    wq[wqi[0] % 2].dma_start(dst3, ones3)
                    wqi[0] += 1
                    i0 += ng

            # ---------------- edge phases ----------------
            ck_of = np.concatenate([[0], np.cumsum(ks.sum(axis=1))]).astype(int)
            for l in range(2):
                tvE, tvO = tviews[l]
                idx = idx1 if l == 0 else idx2
                for w in range(wpc):
                    ck = int(ck_of[w])
                    ke, ko = int(ks[w, 0]), int(ks[w, 1])
                    kw = ke + ko
                    # whole-window index tile + mask prefetch
                    wt = wip.tile([P, kmax * 8], I16, tag="wt", name=f"wt{l}_{w}")
                    if kw:
                        nc.sync.dma_start(wt[:, 0:kw * 8], idx[:, ck * 8:(ck + kw) * 8])
                    mkw = smp.tile([P, kmax], F32, tag="mkw")
                    if kw:
                        nc.sync.dma_start(mkw[:, 0:kw], mk[:, ck:ck + kw])
                    nst = smp.tile([P, HEADS], F32, tag="nst")
                    nc.sync.dma_start(nst[:], nsf[w * P:(w + 1) * P, :])

                    # ---- self-loop contribution + er for this window ----
                    if l == 0:
                        sf = fpsum.tile([P, 268], F32, tag="sf")
                        nc.tensor.matmul(sf[:], lhsT=xst[0:D, w * P:(w + 1) * P],
                                         rhs=w1t[0:D, :], start=True, stop=True)
                        ss = stp.tile([P, 264], BF16, tag="ss")
                        nc.scalar.activation(ss[:], sf[:, 0:264], AF.Copy)
                        nc.vector.memset(ss[:, FD:FD + 4], 1.0)
                        erw = smp.tile([P, HEADS], F32, tag="erw")
                        nc.vector.tensor_copy(erw[:], sf[:, 264:268])
                        esum = smp.tile([P, HEADS], F32, tag="esum")
                        nc.vector.tensor_add(esum[:], sf[:, 260:264], erw[:])
                    else:
                        ss = stp.tile([P, 264], BF16, tag="ss")
                        ci2, ri2 = w // (wpc // NCHUNK), w % (wpc // NCHUNK)
                        nc.sync.dma_start(ss[:], tab_locs1[ci2][ri2 * P:(ri2 + 1) * P, 0:264])
                        erw = smp.tile([P, HEADS], F32, tag="erw")
                        nc.sync.dma_start(erw[:], er_tab2[w * P:(w + 1) * P, :])
                        esum = smp.tile([P, HEADS], F32, tag="esum")
                        nc.vector.tensor_add(esum[:], ss[:, 260:264], erw[:])
                    lrs = smp.tile([P, HEADS], F32, tag="lrs")
                    nc.vector.scalar_tensor_tensor(lrs[:], esum[:], NEG_SLOPE,
                                                   esum[:], op0=ALU.mult, op1=ALU.max)
                    ees = smp.tile([P, HEADS], F32, tag="ees")
                    nc.scalar.activation(ees[:], lrs[:], AF.Exp)
                    nc.vector.tensor_mul(ees[:], ees[:], nst[:])
                    acc = accp.tile([P, 260], BF16, tag="acc")
                    ee_rep = (ees[:].rearrange("p (o h) -> p o h", o=1)
                              .broadcast_to([P, 65, HEADS]))
                    nc.vector.tensor_mul(acc[:].rearrange("p (d h) -> p d h", h=HEADS),
                                         ss[:, 0:260].rearrange("p (d h) -> p d h", h=HEADS),
                                         ee_rep)

                    # ---- gathered-edge passes ----
                    for pi, (a0, a1, b0, b1) in enumerate(passes[w]):
                        kpa, kpb = a1 - a0, b1 - b0
                        kp = kpa + kpb
                        if kp == 0:
                            continue
                        G = gatp.tile([P, K_CAP * ROW], BF16, tag="G",
                                      name=f"G_{l}_{w}_{pi}")
                        g3 = G[:].rearrange("p (k r) -> p k r", r=ROW)
                        for si, (kk, c0, tv) in enumerate((
                            (kpa, a0, tvE),
                            (kpb, b0, tvO),
                        )):
                            if kk == 0:
                                continue
                            koff = 0 if si == 0 else kpa
                            woff = c0 if si == 0 else ke + c0
                            k0 = 0
                            while k0 < kk:
                                kc = min(CHUNK, kk - k0)
                                ni = P * kc
                                nc.gpsimd.dma_gather(
                                    g3[:, koff + k0:koff + k0 + kc, :],
                                    tv,
                                    wt[:, (woff + k0) * 8:(woff + k0 + kc) * 8],
                                    ni, ni, ROW,
                                    elem_step=2 * ROW,
                                    single_packet=False,
                                    queue_num=qctr[0] % 4,
                                )
                                qctr[0] += 1
                                k0 += kc

                        # logits e = el + er + mask   [128, kp, 4] f32
                        e = smp.tile([P, kpmax * HEADS], F32, tag="e")
                        e3 = e[:, 0:kp * HEADS].rearrange("p (k h) -> p k h", h=HEADS)
                        el = g3[:, 0:kp, 260:264]
                        er_rep = (erw[:].rearrange("p (o h) -> p o h", o=1)
                                  .broadcast_to([P, kp, HEADS]))
                        nc.vector.tensor_add(e3, el, er_rep)
                        # mask: E part at [a0:a1], O part at [ke+b0:ke+b1] of mkw
                        if kpa == ke and kpb == ko:
                            mkp = mkw  # single pass covers the whole window
                        else:
                            mkp = smp.tile([P, kpmax], F32, tag="mkp")
                            if kpa:
                                nc.vector.tensor_copy(mkp[:, 0:kpa], mkw[:, a0:a1])
                            if kpb:
                                nc.vector.tensor_copy(mkp[:, kpa:kp], mkw[:, ke + b0:ke + b1])
                        mk_rep = (mkp[:, 0:kp].rearrange("p (k o) -> p k o", o=1)
                                  .broadcast_to([P, kp, HEADS]))
                        nc.vector.tensor_add(e3, e3, mk_rep)
                        # ee = exp(lrelu(e))  bf16
                        lr = smp.tile([P, kpmax * HEADS], F32, tag="lr")
                        nc.vector.scalar_tensor_tensor(
                            lr[:, 0:kp * HEADS], e[:, 0:kp * HEADS], NEG_SLOPE,
                            e[:, 0:kp * HEADS], op0=ALU.mult, op1=ALU.max)
                        ee = smp.tile([P, kpmax * HEADS], BF16, tag="ee")
                        nc.scalar.activation(ee[:, 0:kp * HEADS], lr[:, 0:kp * HEADS], AF.Exp)

                        # msg = G * ee_rep, in place (cols 0:260)
                        m4 = g3[:, 0:kp, 0:260].rearrange("p k (d h) -> p k d h", h=HEADS)
                        ee_rep = (ee[:, 0:kp * HEADS]
                                  .rearrange("p (k o h) -> p k o h", o=1, h=HEADS)
                                  .broadcast_to([P, kp, 65, HEADS]))
                        nc.vector.tensor_mul(m4, m4, ee_rep)

                        # sum the kp message slices on the PE (identity matmul
                        # accumulating in PSUM), then fold into acc
                        tps = trpsum.tile([P, 260], F32, tag="tr")
                        for k in range(kp):
                            nc.tensor.matmul(tps[:], lhsT=ident[:],
                                             rhs=g3[:, k, 0:260],
                                             start=(k == 0), stop=(k == kp - 1))
                        nc.vector.tensor_add(acc[:], acc[:], tps[:])

                    # h = agg/den (+ h1) (+ b)
                    r = smp.tile([P, HEADS], BF16, tag="r")
                    nc.vector.reciprocal(r[:], acc[:, FD:FD + 4])
                    r_rep = (r[:].rearrange("p (o h) -> p o h", o=1)
                             .broadcast_to([P, D, HEADS]))
                    if l == 0:
                        h = htp.tile([P, FD], BF16, tag="h")
                        nc.vector.tensor_mul(h[:].rearrange("p (d h) -> p d h", h=HEADS),
                                             acc[:, 0:FD].rearrange("p (d h) -> p d h", h=HEADS), r_rep)
                        nc.vector.tensor_add(h[:], h[:], b1t[:])
                        nc.scalar.dma_start(h_tab[w * P:(w + 1) * P, :], h[:])
                        # feat2 = h @ W2cat
                        pf = fpsum.tile([P, 268], F32, tag="fp")
                        for t in range(2):
                            pt = tpsum.tile([P, P], BF16, tag="tp")
                            nc.tensor.transpose(pt[:], h[:, t * P:(t + 1) * P], ident[:])
                            hT = htp.tile([P, P], BF16, tag="hT")
                            nc.vector.tensor_copy(hT[:], pt[:])
                            nc.tensor.matmul(pf[:], lhsT=hT[:], rhs=w2t[t][:],
                                             start=(t == 0), stop=(t == 1))
                        st = stp.tile([P, 264], BF16, tag="st")
                        nc.scalar.activation(st[:], pf[:, 0:264], AF.Copy)
                        nc.vector.memset(st[:, FD:FD + 4], 1.0)
                        ers = stp.tile([P, HEADS], F32, tag="ers")
                        nc.vector.tensor_copy(ers[:], pf[:, 264:268])
                        ci2, ri2 = w // (wpc // NCHUNK), w % (wpc // NCHUNK)
                        nc.scalar.dma_start(tab_locs1[ci2][ri2 * P:(ri2 + 1) * P, 0:264], st[:])
                        nc.scalar.dma_start(er_tab2[w * P:(w + 1) * P, :], ers[:])
                        if ri2 == wpc // NCHUNK - 1:
                            nc.gpsimd.collective_compute(
                                "AllGather", ALU.bypass,
                                replica_groups=[list(range(CORES))],
                                ins=[tab_locs1[ci2].opt()],
                                outs=[tabs[1][ci2 * gshard:(ci2 + 1) * gshard, :].opt()],
                            )
                    else:
                        h1w = htp.tile([P, FD], BF16, tag="h1w")
                        nc.sync.dma_start(h1w[:], h_tab[w * P:(w + 1) * P, :])
                        h2 = htp.tile([P, FD], F32, tag="h2")
                        nc.vector.tensor_mul(h2[:].rearrange("p (d h) -> p d h", h=HEADS),
                                             acc[:, 0:FD].rearrange("p (d h) -> p d h", h=HEADS), r_rep)
                        nc.vector.tensor_add(h2[:], h2[:], h1w[:])
                        nc.vector.tensor_add(h2[:], h2[:], b2t[:])
                        nc.scalar.dma_start(out[w * P:(w + 1) * P, :], h2[:])

    nc.finalize()
    return nc


def kernel(x, w1, b1, al1, ar1, w2, b2, al2, ar2, src, dst):
    global LAST_EXEC_NS
    _install_profile_hook()

    n_nodes = x.shape[0]
    x = np.asarray(x, dtype=np.float32)
    src = np.asarray(src, dtype=np.int64)
    dst = np.asarray(dst, dtype=np.int64)

    pp = _prep(src, dst, n_nodes)
    ks, wpc, shard = pp["ks"], pp["wpc"], pp["shard"]

    dm = _dmaj(FD)
    # W1cat [128, 268] bf16: rows 0:64 = [w1_dmaj | 0 | w1al | w1ar]
    w1d = np.asarray(w1, np.float32)[:, dm]
    al1 = np.asarray(al1, np.float32)
    ar1 = np.asarray(ar1, np.float32)
    w1r = np.asarray(w1, np.float32).reshape(D, HEADS, D)
    w1al = np.einsum("khd,hd->kh", w1r, al1)
    w1ar = np.einsum("khd,hd->kh", w1r, ar1)
    w1c = np.zeros((P, 268), np.float32)
    w1c[0:D, 0:FD] = w1d
    w1c[0:D, 260:264] = w1al
    w1c[0:D, 264:268] = w1ar
    w1c = w1c.astype(ml_dtypes.bfloat16)

    al2 = np.asarray(al2, np.float32)
    ar2 = np.asarray(ar2, np.float32)
    w2f = np.asarray(w2, np.float32)
    w2p = w2f[dm][:, dm]
    w2r = w2f[dm].reshape(FD, HEADS, D)
    w2al = np.einsum("khd,hd->kh", w2r, al2)
    w2ar = np.einsum("khd,hd->kh", w2r, ar2)
    w2c = np.zeros((2, P, 268), np.float32)
    for t in range(2):
        w2c[t, :, 0:FD] = w2p[t * P:(t + 1) * P]
        w2c[t, :, 260:264] = w2al[t * P:(t + 1) * P]
        w2c[t, :, 264:268] = w2ar[t * P:(t + 1) * P]
    w2c = w2c.astype(ml_dtypes.bfloat16)

    b1d = np.asarray(b1, np.float32)[dm]
    b2d = np.asarray(b2, np.float32)[dm]
    b1t = np.tile(b1d, (P, 1)).astype(ml_dtypes.bfloat16)
    b2t = np.tile(b2d, (P, 1)).astype(np.float32)

    # xT by layer-1 table row (identical on all cores); x_selfT per core
    rho = pp["rho"]
    full_rows = CORES * shard
    xta = np.zeros((D, full_rows), np.float32)
    xta[:, rho] = x.T
    xta = xta.astype(ml_dtypes.bfloat16)

    in_maps = []
    for c in range(CORES):
        win = pp["core_nodes"][c]  # [wpc, P] node ids, -1 padded
        flat = win.reshape(-1)
        valid = flat >= 0
        xsf = np.zeros((D, shard), np.float32)
        xsf[:, np.nonzero(valid)[0]] = x[flat[valid]].T
        in_maps.append({
            "xta": xta, "xsf": xsf.astype(ml_dtypes.bfloat16),
            "w1c": w1c, "w2c": w2c, "b1b": b1t, "b2b": b2t,
            "idx1": pp["idx1"][c], "idx2": pp["idx2"][c],
            "mk": pp["masks"][c], "nsf": pp["nselfw"][c],
        })

    nc = _build(ks, pp["passes"], wpc, shard, pp["csz"])
    res = run_bass_kernel_spmd(nc, in_maps, core_ids=list(range(CORES)))
    LAST_EXEC_NS = res.exec_time_ns

    inv = np.empty(FD, np.int64)
    inv[dm] = np.arange(FD)
    outf = np.empty((n_nodes, FD), np.float32)
    for c in range(CORES):
        flat = pp["core_nodes"][c].reshape(-1)
        valid = flat >= 0
        sh = res.results[c]["out"]
        outf[flat[valid]] = sh[np.nonzero(valid)[0]][:, inv]
    return outf


# revision 37
# speedup vs baseline: 1.0162x; 1.0162x over previous
"""2-layer GAT (DGL GATConv-style) on 8 TRN2 NeuronCores.

Strategy (host preprocessing is index/structure only; every FLOP that depends
on float inputs runs on device):
 - Nodes are dealt to 8 cores snake-wise by in-degree.  Self-loop edges are
   removed from the gather grid entirely (served by a per-window matmul from
   a per-core x_selfT input for layer 1, and local staged rows for layer 2).
 - Each node gets a parity (even/odd table position), chosen by local search
   so that every dst's in-edges split evenly between parities.  Gathers index
   PAIR-rows (stride 1536B, even/odd base offset), so the full 50k-node table
   fits the int16 index range without an A/B owner split.  Per-core windows
   are packed by max(e,o) so per-window slice counts are tight (pad ~1.14 vs
   1.41 for the old A/B scheme).
 - Layer-1 feature table is built locally on every core from a replicated
   xT input (no layer-1 AllGather).  The layer-2 table AllGather is split
   into 7 chunks overlapped with the layer-1 edge phase.
 - Table row: [256 feat dmaj | 4 ones | 4 el(bf16)] padded to 384 bf16 slots
   (768B, the 256B-granule ceiling).  Denominators ride along as the ones
   columns; el is consumed straight from the gathered rows.
 - Gather calls cover up to 12 slices (1536 rows) on 4 SWDGE queues
   round-robin with 4-deep G buffering to keep the dispatch pipes full.
"""
import sys
import types

import numpy as np
import ml_dtypes

import concourse.bass as bass
import concourse.bacc as bacc
import concourse.tile as tile
from concourse import mybir
from concourse.bass_utils import run_bass_kernel_spmd
from concourse.masks import make_identity

AF = mybir.ActivationFunctionType
ALU = mybir.AluOpType
BF16 = mybir.dt.bfloat16
F32 = mybir.dt.float32
I16 = mybir.dt.int16

P = 128
HEADS = 4
D = 64
FD = HEADS * D          # 256
ROW = 384               # bf16 slots per table row: 256 feat | 4 ones | 4 el | 120 pad
CORES = 8
NEG_SLOPE = 0.2
K_CAP = 24              # max slices per pass (SBUF bound)
CHUNK = 12              # slices per dma_gather call
NCHUNK = 7              # layer-2 AllGather chunks (must divide wpc)

LAST_EXEC_NS = None


def _install_profile_hook():
    """Best-effort NTFF profiling hook (axon images lack antenv.axon_hooks)."""
    try:
        import antenv
        try:
            import antenv.axon_hooks  # noqa: F401
            return
        except ImportError:
            pass
        mod = types.ModuleType("antenv.axon_hooks")
        mod._HOOK = None

        def set_hook(h):
            mod._HOOK = h

        def get_hook():
            return mod._HOOK

        mod.set_axon_ntff_profile_hook = set_hook
        mod.get_axon_ntff_profile_hook = get_hook
        sys.modules["antenv.axon_hooks"] = mod
        antenv.axon_hooks = mod
        from trn_agent_boot.trn_boot import _ntff_profile_via_ctypes
        set_hook(_ntff_profile_via_ctypes("/opt/axon/libaxon_pjrt.so"))
    except Exception:
        pass


def _dmaj(n):
    """column permutation h*64+d -> d*4+h (applied to axis of size 256)."""
    j = np.arange(n)
    d, h = j // HEADS, j % HEADS
    return h * D + d


def _wrap_idx(flat):
    """[128] int16 -> [128, 8] wrapped+replicated for dma_gather."""
    w = flat.reshape(8, 16).T  # [16, 8]
    return np.tile(w, (8, 1)).astype(np.int16)


def _prep(src, dst, n_nodes):
    """Host-side graph preprocessing: parity assignment, window packing,
    per-core index/mask buffers, per-layer row maps."""
    loop = src == dst
    nself = np.bincount(dst[loop], minlength=n_nodes)
    src_e, dst_e = src[~loop], dst[~loop]
    deg = np.bincount(dst_e, minlength=n_nodes)

    order = np.argsort(-deg, kind="stable")
    owner = np.empty(n_nodes, dtype=np.int32)
    pat = np.concatenate([np.arange(CORES), np.arange(CORES)[::-1]])
    owner[order] = pat[np.arange(n_nodes) % (2 * CORES)]

    npc = (n_nodes + CORES - 1) // CORES
    wpc = (npc + P - 1) // P
    assert wpc % NCHUNK == 0, wpc
    shard = wpc * P

    # ---- parity local search: balance each dst's in-edges over parities ----
    so = np.argsort(src_e, kind="stable")
    sstarts = np.zeros(n_nodes + 1, np.int64)
    np.cumsum(np.bincount(src_e, minlength=n_nodes), out=sstarts[1:])
    odf = dst_e[so]

    par = np.zeros(n_nodes, np.int8)
    for c in range(CORES):
        nodes = np.where(owner == c)[0]
        par[nodes] = np.arange(len(nodes)) % 2
    ecnt = np.bincount(dst_e, weights=par[src_e], minlength=n_nodes).astype(np.int64)

    def gain_of(n):
        ds = odf[sstarts[n]:sstarts[n + 1]]
        e, d = ecnt[ds], deg[ds]
        pm = 1 - 2 * int(par[n])
        return (np.maximum(e, d - e) - np.maximum(e + pm, d - e - pm)).sum()

    for sweep in range(10):
        e_of = ecnt[odf]
        d_of = deg[odf]
        pm = (1 - 2 * par[src_e[so]]).astype(np.int64)
        gsnap = np.zeros(n_nodes)
        np.add.at(gsnap, src_e[so], (np.maximum(e_of, d_of - e_of) -
                                     np.maximum(e_of + pm, d_of - e_of - pm)).astype(np.float64))
        swaps = 0
        for c in range(CORES):
            nodes = np.where(owner == c)[0]
            ones = nodes[par[nodes] == 1]
            zeros = nodes[par[nodes] == 0]
            o1 = ones[np.argsort(-gsnap[ones])]
            o0 = zeros[np.argsort(-gsnap[zeros])]
            i = j = 0
            while i < len(o1) and j < len(o0):
                u, v = o1[i], o0[j]
                gu, gv = gain_of(u), gain_of(v)
                if gu + gv <= 0:
                    if gsnap[u] <= gsnap[v]:
                        i += 1
                    else:
                        j += 1
                    if gsnap[u] <= 0 and gsnap[v] <= 0:
                        break
                    continue
                np.add.at(ecnt, odf[sstarts[u]:sstarts[u + 1]], -1)
                par[u] = 0
                np.add.at(ecnt, odf[sstarts[v]:sstarts[v + 1]], 1)
                par[v] = 1
                swaps += 1
                i += 1
                j += 1
        if swaps < 50:
            break

    # ---- window packing: per core sorted by max(e,o), 64/64 parity slots ----
    # partition v within a window is parity-interleaved (par-0 at even v), so
    # table position == w*128+v everywhere (device stages rows by partition).
    core_nodes = []           # per core: window-major list, padded with -1
    pos = np.full(n_nodes, -1, np.int64)
    KEs = np.zeros(wpc, np.int64)
    KOs = np.zeros(wpc, np.int64)
    for c in range(CORES):
        nodes = np.where(owner == c)[0]
        key = np.maximum(ecnt[nodes], deg[nodes] - ecnt[nodes])
        nodes = nodes[np.argsort(key, kind="stable")]
        win = np.full((wpc, P), -1, np.int64)
        wcap = np.full((wpc, 2), 64, np.int64)
        ptr = [0, 0]
        for n in nodes:
            p = int(par[n])
            while wcap[ptr[p], p] == 0:
                ptr[p] += 1
            w = ptr[p]
            v = 2 * (64 - wcap[w, p]) + p
            wcap[w, p] -= 1
            win[w, v] = n
            pos[n] = w * P + v
            KEs[w] = max(KEs[w], ecnt[n])
            KOs[w] = max(KOs[w], deg[n] - ecnt[n])
        core_nodes.append(win)

    rho = pos + owner.astype(np.int64) * shard        # layer-1 table row
    # layer-2 table row (chunk-major for chunked AllGather)
    csz = (wpc // NCHUNK) * P                         # rows per core per chunk
    chn = pos // csz
    rho2 = chn * (CORES * csz) + owner.astype(np.int64) * csz + pos % csz

    ks = np.stack([KEs, KOs], axis=1)
    passes = []
    for w in range(wpc):
        ke, ko = int(KEs[w]), int(KOs[w])
        cuts = []
        a0 = b0 = 0
        while a0 < ke or b0 < ko or not cuts:
            ta = min(ke - a0, K_CAP)
            tb = min(ko - b0, K_CAP - ta)
            cuts.append((a0, a0 + ta, b0, b0 + tb))
            a0 += ta
            b0 += tb
        passes.append(cuts)

    # ---- per-core slot buffers ----
    eo = np.argsort(dst_e, kind="stable")
    starts = np.zeros(n_nodes + 1, np.int64)
    np.cumsum(np.bincount(dst_e, minlength=n_nodes), out=starts[1:])

    sum_k = int((KEs + KOs).sum())
    idx1 = np.zeros((CORES, P, sum_k * 8), np.int16)
    idx2 = np.zeros((CORES, P, sum_k * 8), np.int16)
    masks = np.full((CORES, P, sum_k), -1e30, np.float32)
    # padded partitions get a phantom self-loop so their denominator is 1:
    # keeps h finite there (the cross-partition PE tree sum would otherwise
    # smear their NaNs over the whole window); host discards those rows.
    nselfw = np.ones((CORES, shard, HEADS), np.float32)
    for c in range(CORES):
        win = core_nodes[c]
        buf1 = np.zeros((sum_k, P), np.int16)
        buf2 = np.zeros((sum_k, P), np.int16)
        ck = 0
        for w in range(wpc):
            ke, ko = int(KEs[w]), int(KOs[w])
            for v in range(P):
                n = win[w, v]
                if n < 0:
                    continue
                nselfw[c, w * P + v, :] = nself[n]
                es = eo[starts[n]:starts[n + 1]]
                s = src_e[es]
                se = s[par[s] == 0]
                so_ = s[par[s] == 1]
                r1e, r1o = rho[se] // 2, rho[so_] // 2
                r2e, r2o = rho2[se] // 2, rho2[so_] // 2
                buf1[ck:ck + len(r1e), v] = r1e
                buf1[ck + ke:ck + ke + len(r1o), v] = r1o
                buf2[ck:ck + len(r2e), v] = r2e
                buf2[ck + ke:ck + ke + len(r2o), v] = r2o
                masks[c, v, ck:ck + len(r1e)] = 0.0
                masks[c, v, ck + ke:ck + ke + len(r1o)] = 0.0
            ck += ke + ko
        idx1[c] = np.concatenate([_wrap_idx(buf1[i]) for i in range(sum_k)], axis=1)
        idx2[c] = np.concatenate([_wrap_idx(buf2[i]) for i in range(sum_k)], axis=1)

    return dict(ks=ks, wpc=wpc, shard=shard, passes=passes, csz=csz,
                core_nodes=core_nodes, rho=rho, idx1=idx1, idx2=idx2,
                masks=masks, nselfw=nselfw, sum_k=sum_k)


def _build(ks, passes, wpc, shard, csz):
    """Build the SPMD bass program (identical on all cores)."""
    sum_k = int(ks.sum())
    kmax = int(ks.sum(axis=1).max())
    kpmax = max(max(a1 - a0 + b1 - b0 for (a0, a1, b0, b1) in cuts) for cuts in passes)
    full_rows = CORES * shard
    gshard = CORES * csz  # rows per chunk in the layer-2 table

    nc = bacc.Bacc("TRN2", target_bir_lowering=False, num_swdge_queues=4,
                   num_devices=CORES, dynamic_dma_scratch_size=16384)
    xta = nc.dram_tensor("xta", [D, full_rows], BF16, kind="ExternalInput")
    xsf = nc.dram_tensor("xsf", [D, shard], BF16, kind="ExternalInput")
    w1c = nc.dram_tensor("w1c", [P, 268], BF16, kind="ExternalInput")
    w2c = nc.dram_tensor("w2c", [2, P, 268], BF16, kind="ExternalInput")
    b1b = nc.dram_tensor("b1b", [P, FD], BF16, kind="ExternalInput")
    b2b = nc.dram_tensor("b2b", [P, FD], F32, kind="ExternalInput")
    idx1 = nc.dram_tensor("idx1", [P, sum_k * 8], I16, kind="ExternalInput")
    idx2 = nc.dram_tensor("idx2", [P, sum_k * 8], I16, kind="ExternalInput")
    mk = nc.dram_tensor("mk", [P, sum_k], F32, kind="ExternalInput")
    nsf = nc.dram_tensor("nsf", [shard, HEADS], F32, kind="ExternalInput")
    out = nc.dram_tensor("out", [shard, FD], F32, kind="ExternalOutput")

    qctr = [0]

    with tile.TileContext(nc) as tc, nc.allow_low_precision(reason="bf16 message accumulation is within tolerance"):
        with (
            tc.tile_pool(name="const", bufs=1) as cpool,
            tc.tile_pool(name="xt", bufs=2) as xtp,
            tc.tile_pool(name="sgrp", bufs=2) as sgp,
            tc.tile_pool(name="fpsum", bufs=3, space="PSUM") as fpsum,
            tc.tile_pool(name="sfpsum", bufs=1, space="PSUM") as sfpsum,
            tc.tile_pool(name="tpsum", bufs=2, space="PSUM") as tpsum,
            tc.tile_pool(name="stage", bufs=3) as stp,
            tc.tile_pool(name="gat", bufs=5) as gatp,
            tc.tile_pool(name="trp", bufs=2, space="PSUM") as trpsum,
            tc.tile_pool(name="accp", bufs=2) as accp,
            tc.tile_pool(name="small", bufs=8) as smp,
            tc.tile_pool(name="widx", bufs=3) as wip,
            tc.tile_pool(name="ht", bufs=2) as htp,
            tc.tile_pool(name="dram", bufs=1, space="DRAM") as dram,
        ):
            ident = cpool.tile([P, P], BF16)
            make_identity(nc, ident[:])
            w1t = cpool.tile([P, 268], BF16)
            nc.sync.dma_start(w1t[:], w1c[:])
            w2t = [cpool.tile([P, 268], BF16, tag=f"w2_{i}", name=f"w2t{i}") for i in range(2)]
            nc.sync.dma_start(w2t[0][:], w2c[0])
            nc.sync.dma_start(w2t[1][:], w2c[1])
            b1t = cpool.tile([P, FD], BF16)
            nc.sync.dma_start(b1t[:], b1b[:])
            b2t = cpool.tile([P, FD], F32)
            nc.sync.dma_start(b2t[:], b2b[:])
            xst = cpool.tile([D, shard], BF16)
            nc.sync.dma_start(xst[:], xsf[:])

            tabs = [dram.tile([full_rows, ROW], BF16, tag=f"tab{l}", name=f"tab{l}")
                    for l in range(2)]
            tab_locs1 = [dram.tile([csz, ROW], BF16, tag=f"tl{i}", name=f"tl{i}")
                         for i in range(NCHUNK)]
            er_tab2 = dram.tile([shard, HEADS], F32)
            h_tab = dram.tile([shard, FD], BF16)

            # pair-row views of the tables (stride 1536B, even/odd base)
            tviews = []
            for l in range(2):
                t2 = tabs[l][:].rearrange("(r two) c -> r (two c)", two=2)
                tviews.append((t2[:, 0:ROW], t2[:, ROW:2 * ROW]))

            # ---------------- layer-1 table build (local, full table) --------
            GRP = [25, 24]  # chunks per write group within each shard
            wq = [nc.sync, nc.scalar]
            wqi = [0]
            for g in range(CORES):
                xt = xtp.tile([D, shard], BF16, tag="xt")
                nc.sync.dma_start(xt[:], xta[:, g * shard:(g + 1) * shard])
                i0 = 0
                for gi, ng in enumerate(GRP):
                    sg = sgp.tile([P, 25 * 264], BF16, tag="sg")
                    for i in range(ng):
                        ci = i0 + i
                        pf = fpsum.tile([P, 268], F32, tag="fp")
                        nc.tensor.matmul(pf[:], lhsT=xt[0:D, ci * P:(ci + 1) * P],
                                         rhs=w1t[0:D, :], start=True, stop=True)
                        if ci % 2:
                            nc.scalar.activation(sg[:, i * 264:i * 264 + 264],
                                                 pf[:, 0:264], AF.Copy)
                        else:
                            nc.vector.tensor_copy(sg[:, i * 264:i * 264 + 264],
                                                  pf[:, 0:264])
                    ones3 = sg[:, 0:ng * 264].rearrange("p (k r) -> p k r", r=264)
                    nc.vector.memset(ones3[:, :, FD:FD + 4], 1.0)
                    # write group -> tab rows [g*shard + i0*P, +ng*P), cols 0:264
                    dst3 = (tabs[0][g * shard + i0 * P:g * shard + (i0 + ng) * P, 0:264]
                            .rearrange("(k p) c -> p k c", p=P))
                    wq[wqi[0] % 2].dma_start(dst3, ones3)
                    wqi[0] += 1
                    i0 += ng

            # ---------------- edge phases ----------------
            ck_of = np.concatenate([[0], np.cumsum(ks.sum(axis=1))]).astype(int)
            for l in range(2):
                tvE, tvO = tviews[l]
                idx = idx1 if l == 0 else idx2
                for w in reversed(range(wpc)):
                    ck = int(ck_of[w])
                    ke, ko = int(ks[w, 0]), int(ks[w, 1])
                    kw = ke + ko
                    # whole-window index tile + mask prefetch
                    wt = wip.tile([P, kmax * 8], I16, tag="wt", name=f"wt{l}_{w}")
                    if kw:
                        nc.sync.dma_start(wt[:, 0:kw * 8], idx[:, ck * 8:(ck + kw) * 8])
                    mkw = smp.tile([P, kmax], F32, tag="mkw")
                    if kw:
                        nc.sync.dma_start(mkw[:, 0:kw], mk[:, ck:ck + kw])
                    nst = smp.tile([P, HEADS], F32, tag="nst")
                    nc.sync.dma_start(nst[:], nsf[w * P:(w + 1) * P, :])

                    # ---- self-loop contribution + er for this window ----
                    if l == 0:
                        sf = sfpsum.tile([P, 268], F32, tag="sf")
                        nc.tensor.matmul(sf[:], lhsT=xst[0:D, w * P:(w + 1) * P],
                                         rhs=w1t[0:D, :], start=True, stop=True)
                        ss = stp.tile([P, 264], BF16, tag="ss")
                        nc.scalar.activation(ss[:], sf[:, 0:264], AF.Copy)
                        nc.vector.memset(ss[:, FD:FD + 4], 1.0)
                        erw = smp.tile([P, HEADS], F32, tag="erw")
                        nc.vector.tensor_copy(erw[:], sf[:, 264:268])
                        esum = smp.tile([P, HEADS], F32, tag="esum")
                        nc.vector.tensor_add(esum[:], sf[:, 260:264], erw[:])
                    else:
                        ss = stp.tile([P, 264], BF16, tag="ss")
                        ci2, ri2 = w // (wpc // NCHUNK), w % (wpc // NCHUNK)
                        nc.sync.dma_start(ss[:], tab_locs1[ci2][ri2 * P:(ri2 + 1) * P, 0:264])
                        erw = smp.tile([P, HEADS], F32, tag="erw")
                        nc.sync.dma_start(erw[:], er_tab2[w * P:(w + 1) * P, :])
                        esum = smp.tile([P, HEADS], F32, tag="esum")
                        nc.vector.tensor_add(esum[:], ss[:, 260:264], erw[:])
                    lrs = smp.tile([P, HEADS], F32, tag="lrs")
                    nc.vector.scalar_tensor_tensor(lrs[:], esum[:], NEG_SLOPE,
                                                   esum[:], op0=ALU.mult, op1=ALU.max)
                    ees = smp.tile([P, HEADS], F32, tag="ees")
                    nc.scalar.activation(ees[:], lrs[:], AF.Exp)
                    nc.vector.tensor_mul(ees[:], ees[:], nst[:])
                    acc = accp.tile([P, 260], BF16, tag="acc")
                    ee_rep = (ees[:].rearrange("p (o h) -> p o h", o=1)
                              .broadcast_to([P, 65, HEADS]))
                    nc.vector.tensor_mul(acc[:].rearrange("p (d h) -> p d h", h=HEADS),
                                         ss[:, 0:260].rearrange("p (d h) -> p d h", h=HEADS),
                                         ee_rep)

                    # ---- gathered-edge passes ----
                    for pi, (a0, a1, b0, b1) in enumerate(passes[w]):
                        kpa, kpb = a1 - a0, b1 - b0
                        kp = kpa + kpb
                        if kp == 0:
                            continue
                        G = gatp.tile([P, K_CAP * ROW], BF16, tag="G",
                                      name=f"G_{l}_{w}_{pi}")
                        g3 = G[:].rearrange("p (k r) -> p k r", r=ROW)
                        for si, (kk, c0, tv) in enumerate((
                            (kpa, a0, tvE),
                            (kpb, b0, tvO),
                        )):
                            if kk == 0:
                                continue
                            koff = 0 if si == 0 else kpa
                            woff = c0 if si == 0 else ke + c0
                            k0 = 0
                            while k0 < kk:
                                kc = min(CHUNK, kk - k0)
                                ni = P * kc
                                nc.gpsimd.dma_gather(
                                    g3[:, koff + k0:koff + k0 + kc, :],
                                    tv,
                                    wt[:, (woff + k0) * 8:(woff + k0 + kc) * 8],
                                    ni, ni, ROW,
                                    elem_step=2 * ROW,
                                    single_packet=False,
                                    queue_num=qctr[0] % 4,
                                )
                                qctr[0] += 1
                                k0 += kc

                        # logits e = el + er + mask   [128, kp, 4] f32
                        e = smp.tile([P, kpmax * HEADS], F32, tag="e")
                        e3 = e[:, 0:kp * HEADS].rearrange("p (k h) -> p k h", h=HEADS)
                        el = g3[:, 0:kp, 260:264]
                        er_rep = (erw[:].rearrange("p (o h) -> p o h", o=1)
                                  .broadcast_to([P, kp, HEADS]))
                        nc.vector.tensor_add(e3, el, er_rep)
                        # mask: E part at [a0:a1], O part at [ke+b0:ke+b1] of mkw
                        if kpa == ke and kpb == ko:
                            mkp = mkw  # single pass covers the whole window
                        else:
                            mkp = smp.tile([P, kpmax], F32, tag="mkp")
                            if kpa:
                                nc.vector.tensor_copy(mkp[:, 0:kpa], mkw[:, a0:a1])
                            if kpb:
                                nc.vector.tensor_copy(mkp[:, kpa:kp], mkw[:, ke + b0:ke + b1])
                        mk_rep = (mkp[:, 0:kp].rearrange("p (k o) -> p k o", o=1)
                                  .broadcast_to([P, kp, HEADS]))
                        nc.vector.tensor_add(e3, e3, mk_rep)
                        # ee = exp(lrelu(e))  bf16
                        lr = smp.tile([P, kpmax * HEADS], F32, tag="lr")
                        nc.vector.scalar_tensor_tensor(
                            lr[:, 0:kp * HEADS], e[:, 0:kp * HEADS], NEG_SLOPE,
                            e[:, 0:kp * HEADS], op0=ALU.mult, op1=ALU.max)
                        ee = smp.tile([P, kpmax * HEADS], BF16, tag="ee")
                        nc.scalar.activation(ee[:, 0:kp * HEADS], lr[:, 0:kp * HEADS], AF.Exp)

                        # msg = G * ee_rep, in place (cols 0:260)
                        m4 = g3[:, 0:kp, 0:260].rearrange("p k (d h) -> p k d h", h=HEADS)
                        ee_rep = (ee[:, 0:kp * HEADS]
                                  .rearrange("p (k o h) -> p k o h", o=1, h=HEADS)
                                  .broadcast_to([P, kp, 65, HEADS]))
                        nc.vector.tensor_mul(m4, m4, ee_rep)

                        # sum the kp message slices on the PE (identity matmul
                        # accumulating in PSUM), then fold into acc
                        tps = trpsum.tile([P, 260], F32, tag="tr")
                        for k in range(kp):
                            nc.tensor.matmul(tps[:], lhsT=ident[:],
                                             rhs=g3[:, k, 0:260],
                                             start=(k == 0), stop=(k == kp - 1))
                        nc.vector.tensor_add(acc[:], acc[:], tps[:])

                    # h = agg/den (+ h1) (+ b)
                    r = smp.tile([P, HEADS], BF16, tag="r")
                    nc.vector.reciprocal(r[:], acc[:, FD:FD + 4])
                    r_rep = (r[:].rearrange("p (o h) -> p o h", o=1)
                             .broadcast_to([P, D, HEADS]))
                    if l == 0:
                        h = htp.tile([P, FD], BF16, tag="h")
                        nc.vector.tensor_mul(h[:].rearrange("p (d h) -> p d h", h=HEADS),
                                             acc[:, 0:FD].rearrange("p (d h) -> p d h", h=HEADS), r_rep)
                        nc.vector.tensor_add(h[:], h[:], b1t[:])
                        nc.scalar.dma_start(h_tab[w * P:(w + 1) * P, :], h[:])
                        # feat2 = h @ W2cat
                        pf = fpsum.tile([P, 268], F32, tag="fp")
                        for t in range(2):
                            pt = tpsum.tile([P, P], BF16, tag="tp")
                            nc.tensor.transpose(pt[:], h[:, t * P:(t + 1) * P], ident[:])
                            hT = htp.tile([P, P], BF16, tag="hT")
                            nc.vector.tensor_copy(hT[:], pt[:])
                            nc.tensor.matmul(pf[:], lhsT=hT[:], rhs=w2t[t][:],
                                             start=(t == 0), stop=(t == 1))
                        st = stp.tile([P, 264], BF16, tag="st")
                        nc.scalar.activation(st[:], pf[:, 0:264], AF.Copy)
                        nc.vector.memset(st[:, FD:FD + 4], 1.0)
                        ers = stp.tile([P, HEADS], F32, tag="ers")
                        nc.vector.tensor_copy(ers[:], pf[:, 264:268])
                        ci2, ri2 = w // (wpc // NCHUNK), w % (wpc // NCHUNK)
                        nc.scalar.dma_start(tab_locs1[ci2][ri2 * P:(ri2 + 1) * P, 0:264], st[:])
                        nc.scalar.dma_start(er_tab2[w * P:(w + 1) * P, :], ers[:])
                        if ri2 == 0:  # reversed order: chunk completes at its lowest window
                            nc.gpsimd.collective_compute(
                                "AllGather", ALU.bypass,
                                replica_groups=[list(range(CORES))],
                                ins=[tab_locs1[ci2].opt()],
                                outs=[tabs[1][ci2 * gshard:(ci2 + 1) * gshard, :].opt()],
                            )
                    else:
                        h1w = htp.tile([P, FD], BF16, tag="h1w")
                        nc.sync.dma_start(h1w[:], h_tab[w * P:(w + 1) * P, :])
                        h2 = htp.tile([P, FD], F32, tag="h2")
                        nc.vector.tensor_mul(h2[:].rearrange("p (d h) -> p d h", h=HEADS),
                                             acc[:, 0:FD].rearrange("p (d h) -> p d h", h=HEADS), r_rep)
                        nc.vector.tensor_add(h2[:], h2[:], h1w[:])
                        nc.vector.tensor_add(h2[:], h2[:], b2t[:])
                        nc.scalar.dma_start(out[w * P:(w + 1) * P, :], h2[:])

    nc.finalize()
    return nc


def kernel(x, w1, b1, al1, ar1, w2, b2, al2, ar2, src, dst):
    global LAST_EXEC_NS
    _install_profile_hook()

    n_nodes = x.shape[0]
    x = np.asarray(x, dtype=np.float32)
    src = np.asarray(src, dtype=np.int64)
    dst = np.asarray(dst, dtype=np.int64)

    pp = _prep(src, dst, n_nodes)
    ks, wpc, shard = pp["ks"], pp["wpc"], pp["shard"]

    dm = _dmaj(FD)
    # W1cat [128, 268] bf16: rows 0:64 = [w1_dmaj | 0 | w1al | w1ar]
    w1d = np.asarray(w1, np.float32)[:, dm]
    al1 = np.asarray(al1, np.float32)
    ar1 = np.asarray(ar1, np.float32)
    w1r = np.asarray(w1, np.float32).reshape(D, HEADS, D)
    w1al = np.einsum("khd,hd->kh", w1r, al1)
    w1ar = np.einsum("khd,hd->kh", w1r, ar1)
    w1c = np.zeros((P, 268), np.float32)
    w1c[0:D, 0:FD] = w1d
    w1c[0:D, 260:264] = w1al
    w1c[0:D, 264:268] = w1ar
    w1c = w1c.astype(ml_dtypes.bfloat16)

    al2 = np.asarray(al2, np.float32)
    ar2 = np.asarray(ar2, np.float32)
    w2f = np.asarray(w2, np.float32)
    w2p = w2f[dm][:, dm]
    w2r = w2f[dm].reshape(FD, HEADS, D)
    w2al = np.einsum("khd,hd->kh", w2r, al2)
    w2ar = np.einsum("khd,hd->kh", w2r, ar2)
    w2c = np.zeros((2, P, 268), np.float32)
    for t in range(2):
        w2c[t, :, 0:FD] = w2p[t * P:(t + 1) * P]
        w2c[t, :, 260:264] = w2al[t * P:(t + 1) * P]
        w2c[t, :, 264:268] = w2ar[t * P:(t + 1) * P]
    w2c = w2c.astype(ml_dtypes.bfloat16)

    b1d = np.asarray(b1, np.float32)[dm]
    b2d = np.asarray(b2, np.float32)[dm]
    b1t = np.tile(b1d, (P, 1)).astype(ml_dtypes.bfloat16)
    b2t = np.tile(b2d, (P, 1)).astype(np.float32)

    # xT by layer-1 table row (identical on all cores); x_selfT per core
    rho = pp["rho"]
    full_rows = CORES * shard
    xta = np.zeros((D, full_rows), np.float32)
    xta[:, rho] = x.T
    xta = xta.astype(ml_dtypes.bfloat16)

    in_maps = []
    for c in range(CORES):
        win = pp["core_nodes"][c]  # [wpc, P] node ids, -1 padded
        flat = win.reshape(-1)
        valid = flat >= 0
        xsf = np.zeros((D, shard), np.float32)
        xsf[:, np.nonzero(valid)[0]] = x[flat[valid]].T
        in_maps.append({
            "xta": xta, "xsf": xsf.astype(ml_dtypes.bfloat16),
            "w1c": w1c, "w2c": w2c, "b1b": b1t, "b2b": b2t,
            "idx1": pp["idx1"][c], "idx2": pp["idx2"][c],
            "mk": pp["masks"][c], "nsf": pp["nselfw"][c],
        })

    nc = _build(ks, pp["passes"], wpc, shard, pp["csz"])
    res = run_bass_kernel_spmd(nc, in_maps, core_ids=list(range(CORES)))
    LAST_EXEC_NS = res.exec_time_ns

    inv = np.empty(FD, np.int64)
    inv[dm] = np.arange(FD)
    outf = np.empty((n_nodes, FD), np.float32)
    for c in range(CORES):
        flat = pp["core_nodes"][c].reshape(-1)
        valid = flat >= 0
        sh = res.results[c]["out"]
        outf[flat[valid]] = sh[np.nonzero(valid)[0]][:, inv]
    return outf
